# revision 33
# baseline (speedup 1.0000x reference)
"""Causal multi-head attention layer for Trainium2, sharded over 8 NeuronCores.

Problem: B=2, T=2048, E=1024, H=16 heads (D=64), fp32.
  out = softmax(mask(QK^T)/sqrt(E)) V Wo + bo   with Q=xWq+bq etc.

Sharding: data-parallel over batch (2) x tensor-parallel over head groups (4):
core c -> batch b=c//4, head group g=c%4 (4 heads, 256 channels).
Each core computes partial = attn_heads(x_b) @ Wo[rows of g]; host sums the 4
partials per batch and adds the bias row.

Math folds used (all exact):
 - bk drops out of softmax (additive shift along the softmax axis).
 - attn rows sum to 1  =>  attn @ (V + 1 bv^T) = attn@V + 1 bv^T, so bv enters
   the output as the constant row bv @ Wo, added on the host with bo.
 - bq is added to Q^T on-chip (per-partition bias).

Device layout per core (all matmuls out = lhsT.T @ rhs, contraction on
partitions):
 - host passes x_b^T; projections: QT/KT = W^T x^T (lhsT=W tiles), V = x W
   (lhsT = x^T tiles), all in fp32r (full-rate fp32 matmul mode).
 - energy^T(kt-block, q) = K Q^T per head, two heads row-packed in the
   128x128 PE array (d=64 contraction at base partitions 0/64).
 - U = exp(energy^T/32) on ACT straight out of PSUM, bf16, causal blocks
   only; diagonal blocks masked with a 128x128 triangular multiply.
 - O^T = [V | 1]^T U accumulated over kt in PSUM: rows 0..63 = unnormalized
   output, row 64 = softmax denominator (free).
 - normalize with DVE mult by gpsimd-broadcast reciprocal; odd heads are
   shifted to partitions 64..127 of the O^T store by an SBUF->SBUF DMA.
 - partial = (O^T).T @ Wo rows, PSUM -> SBUF -> DRAM.
"""

import os
import numpy as np

B, T, E, H = 2, 2048, 1024, 16
P = 128
NCORES = 8
G = 4            # head groups (tensor parallel)
HG = H // G      # heads per group = 4
D = E // H       # 64
CW = HG * D      # channels per group = 256
ET = E // P      # 8 e-tiles
MT = CW // P     # 2 hd-tiles
TT = T // P      # 16 t-tiles
NQ = 512         # q-chunk width
QC = T // NQ     # 4 q-chunks
SCALE = 1.0 / np.sqrt(E).astype(np.float32)  # 1/32

_CACHE: dict = {}


def _build_bass(debug_taps=False):
    import concourse.bass as bass
    import concourse.mybir as mybir
    import concourse.tile as tile
    from concourse import bacc

    f32 = mybir.dt.float32
    f32r = mybir.dt.float32r  # fp32 data, full-rate (1 cyc/row) PE matmul mode
    bf16 = mybir.dt.bfloat16
    Exp = mybir.ActivationFunctionType.Exp

    nc = bacc.Bacc("TRN2", target_bir_lowering=False, name="attn_tp")
    dbg = {}
    if debug_taps:
        dbg["qt00"] = nc.dram_tensor("dbg_qt00", [P, NQ], bf16, kind="ExternalOutput")
        dbg["kt00"] = nc.dram_tensor("dbg_kt00", [P, NQ], bf16, kind="ExternalOutput")
        dbg["vo0"] = nc.dram_tensor("dbg_vo0", [P, HG * (D + 1)], bf16, kind="ExternalOutput")
        dbg["u000"] = nc.dram_tensor("dbg_u000", [P, 2 * NQ], bf16, kind="ExternalOutput")
        dbg["oun0"] = nc.dram_tensor("dbg_oun0", [P, NQ], f32, kind="ExternalOutput")
        dbg["bc0"] = nc.dram_tensor("dbg_bc0", [P, NQ], f32, kind="ExternalOutput")
        dbg["ot00"] = nc.dram_tensor("dbg_ot00", [P, NQ], bf16, kind="ExternalOutput")
    xt = nc.dram_tensor("xt", [E, T], bf16, kind="ExternalInput")
    wq = nc.dram_tensor("wq", [P, ET, CW], bf16, kind="ExternalInput")
    wk = nc.dram_tensor("wk", [P, ET, CW], bf16, kind="ExternalInput")
    wv = nc.dram_tensor("wv", [P, ET, CW], bf16, kind="ExternalInput")
    bq = nc.dram_tensor("bq", [P, MT], f32, kind="ExternalInput")
    wo = nc.dram_tensor("wo", [P, MT, E], bf16, kind="ExternalInput")
    tri = nc.dram_tensor("tri", [P, P], bf16, kind="ExternalInput")
    out = nc.dram_tensor("out", [T, E], f32, kind="ExternalOutput")

    with tile.TileContext(nc) as tc:
        with (
            tc.tile_pool(name="persist", bufs=1) as pers,
            tc.tile_pool(name="pp", bufs=2, space="PSUM") as pp,
            tc.tile_pool(name="ep", bufs=2, space="PSUM") as ep,
            tc.tile_pool(name="op", bufs=2, space="PSUM") as op,
            tc.tile_pool(name="up", bufs=8) as up,
            tc.tile_pool(name="sm", bufs=4) as sm,
            tc.tile_pool(name="ost", bufs=6) as ost,
        ):
            # ---- persistent SBUF tensors ----
            xt_t = [pers.tile([P, T], bf16, tag=f"xt{a}", name=f"xt{a}") for a in range(ET)]
            wq_sb = pers.tile([P, ET, CW], bf16, tag="wq_sb", name="wq_sb")
            wk_sb = pers.tile([P, ET, CW], bf16, tag="wk_sb", name="wk_sb")
            wv_sb = pers.tile([P, ET, CW], bf16, tag="wv_sb", name="wv_sb")
            wo_sb = pers.tile([P, MT, E], bf16, tag="wo_sb", name="wo_sb")
            bq_sb = pers.tile([P, MT], f32, tag="bq_sb", name="bq_sb")
            tri_sb = pers.tile([P, P], bf16, tag="tri_sb", name="tri_sb")
            qt_t = [[pers.tile([P, NQ], bf16, tag=f"qt{m}_{n}", name=f"qt{m}_{n}")
                     for n in range(QC)] for m in range(MT)]
            kt_t = [[pers.tile([P, NQ], bf16, tag=f"kt{m}_{n}", name=f"kt{m}_{n}")
                     for n in range(QC)] for m in range(MT)]
            vo_t = [pers.tile([P, HG, D + 1], bf16, tag=f"vo{t}", name=f"vo{t}")
                    for t in range(TT)]
            ot_t = [[pers.tile([P, NQ], bf16, tag=f"ot{m}_{n}", name=f"ot{m}_{n}")
                     for n in range(QC)] for m in range(MT)]

            # ---- input DMAs (weights first so projections start early) ----
            nc.scalar.dma_start(out=wq_sb, in_=wq[:, :, :])
            nc.scalar.dma_start(out=bq_sb, in_=bq[:, :])
            nc.scalar.dma_start(out=wk_sb, in_=wk[:, :, :])
            nc.scalar.dma_start(out=wv_sb, in_=wv[:, :, :])
            nc.scalar.dma_start(out=tri_sb, in_=tri[:, :])
            xt_r = xt.rearrange("(a p) t -> a p t", p=P)
            for a in range(ET):
                nc.sync.dma_start(out=xt_t[a], in_=xt_r[a])
            nc.scalar.dma_start(out=wo_sb, in_=wo[:, :, :])

            # ---- software-pipelined emission ----
            # PE engine queues are in-order, so attention batches (gated on
            # ACT exp) are interleaved with independent filler work: the next
            # chunk's projection groups and the previous chunk's Wo groups.

            def proj_closures(n):
                def qk_group(wsb, m, biased):
                    def f():
                        ps = pp.tile([P, NQ], f32, tag="pp_t", name="psqk")
                        for a in range(ET):
                            nc.tensor.matmul(
                                ps,
                                lhsT=wsb[:, a, m * P:(m + 1) * P],
                                rhs=xt_t[a][:, n * NQ:(n + 1) * NQ],
                                start=(a == 0), stop=(a == ET - 1),
                            )
                        if biased:
                            nc.vector.tensor_scalar_add(
                                out=qt_t[m][n], in0=ps, scalar1=bq_sb[:, m:m + 1])
                        else:
                            nc.vector.tensor_copy(out=kt_t[m][n], in_=ps)
                    return f

                def v_group(t):
                    def f():
                        psv = pp.tile([P, NQ], f32, tag="pp_t", name="psv")
                        for a in range(ET):
                            nc.tensor.matmul(
                                psv[:, :CW],
                                lhsT=xt_t[a][:, t * P:(t + 1) * P],
                                rhs=wv_sb[:, a, :],
                                start=(a == 0), stop=(a == ET - 1),
                            )
                        nc.vector.tensor_copy(
                            out=vo_t[t][:, :, 0:D],
                            in_=psv[:, :CW].rearrange("p (h d) -> p h d", h=HG))
                        nc.vector.memset(vo_t[t][:, :, D:D + 1], 1.0)
                    return f

                fs = []
                for m in range(MT):
                    fs.append(qk_group(wq_sb, m, True))
                    fs.append(qk_group(wk_sb, m, False))
                for t in range(4 * n, 4 * n + 4):
                    fs.append(v_group(t))
                return fs

            def wo_closures(qc):
                def wo_group(ti, ec):
                    def f():
                        wp = pp.tile([P, NQ], f32, tag="pp_t", name="wp")
                        for m in range(MT):
                            nc.tensor.matmul(
                                wp,
                                lhsT=ot_t[m][qc][:, (ti % 4) * P:(ti % 4 + 1) * P],
                                rhs=wo_sb[:, m, ec * NQ:(ec + 1) * NQ],
                                start=(m == 0), stop=(m == MT - 1),
                            )
                        so = ost.tile([P, NQ], f32, tag="ost", name="so")
                        if ec % 2 == 0:
                            nc.scalar.copy(out=so, in_=wp)
                        else:
                            nc.vector.tensor_copy(out=so, in_=wp)
                        nc.sync.dma_start(
                            out=out[ti * P:(ti + 1) * P, ec * NQ:(ec + 1) * NQ], in_=so)
                    return f
                return [wo_group(ti, ec)
                        for ti in range(4 * qc, 4 * qc + 4) for ec in range(E // NQ)]

            def attn_stream(qc):
                nkt = 4 * qc + 4
                for pair in ((0, 1), (2, 3)):
                    o_ps = {}

                    def alloc(pair=pair, o_ps=o_ps):
                        for h in pair:
                            o_ps[h] = op.tile([P, NQ], f32, tag="o_ps", name=f"o_ps{h}")

                    def batch(ktb, pair=pair, o_ps=o_ps):
                        kts = (ktb, ktb + 1)
                        offs = [max(0, (kt - 4 * qc) * P) for kt in kts]
                        e_ts = {}
                        u_ts = {}
                        for h in pair:
                            e_ts[h] = ep.tile([P, 2 * NQ], f32, tag="e_ps",
                                              name=f"e_ps{h}")
                        # alternate heads so adjacent matmuls use disjoint PE
                        # row groups (base partitions 0/64): LDWEIGHTS of the
                        # next matmul overlaps the in-flight one
                        for j, kt in enumerate(kts):
                            eoff = offs[j]
                            for h in pair:
                                m, r0 = h // 2, 64 * (h % 2)
                                nc.tensor.matmul(
                                    e_ts[h][:, j * NQ + eoff:(j + 1) * NQ],
                                    lhsT=kt_t[m][kt // 4][r0:r0 + D,
                                                          (kt % 4) * P:(kt % 4 + 1) * P],
                                    rhs=qt_t[m][qc][r0:r0 + D, eoff:NQ],
                                    start=True, stop=True,
                                )
                        for h in pair:
                            ut = up.tile([P, 2 * NQ], bf16, tag="u", name=f"u{h}")
                            u_ts[h] = ut
                            if offs[1] <= P:
                                # single ACTIVATE; the [NQ, NQ+off1) hole is
                                # never read downstream
                                nc.scalar.activation(
                                    ut[:, offs[0]:], e_ts[h][:, offs[0]:],
                                    Exp, scale=float(SCALE))
                            else:
                                for j, off in enumerate(offs):
                                    nc.scalar.activation(
                                        ut[:, j * NQ + off:(j + 1) * NQ],
                                        e_ts[h][:, j * NQ + off:(j + 1) * NQ],
                                        Exp, scale=float(SCALE))
                            for j, kt in enumerate(kts):
                                if kt >= 4 * qc:
                                    w0 = j * NQ + offs[j]
                                    nc.vector.tensor_mul(
                                        ut[:, w0:w0 + P], ut[:, w0:w0 + P], tri_sb)
                        if debug_taps and qc == 0 and pair == (0, 1) and ktb == 0:
                            nc.sync.dma_start(out=dbg["u000"][:, :], in_=u_ts[0])
                        for h in pair:
                            for j, kt in enumerate(kts):
                                off = offs[j]
                                nc.tensor.matmul(
                                    o_ps[h][0:D + 1, off:NQ],
                                    lhsT=vo_t[kt][:, h, :],
                                    rhs=u_ts[h][:, j * NQ + off:(j + 1) * NQ],
                                    start=(kt == 0), stop=(kt == nkt - 1),
                                )

                    def norm(h, pair=pair, o_ps=o_ps):
                        m, r0 = h // 2, 64 * (h % 2)
                        if debug_taps and qc == 0 and h == 0:
                            ou = sm.tile([P, NQ], f32, tag="oun", name="oun")
                            nc.vector.tensor_copy(out=ou[0:D + 1, :], in_=o_ps[h][0:D + 1, :])
                            nc.sync.dma_start(out=dbg["oun0"][:, :], in_=ou)
                        dn = sm.tile([P, NQ], f32, tag="dn", name="dn")
                        nc.vector.tensor_copy(out=dn[D:D + 1, :], in_=o_ps[h][D:D + 1, :])
                        nc.sync.dma_start(out=dn[0:1, :], in_=dn[D:D + 1, :])
                        rc = sm.tile([P, NQ], f32, tag="rc", name="rc")
                        nc.vector.reciprocal_approx_fast(out=rc[0:1, :], in_=dn[0:1, :])
                        bc = sm.tile([P, NQ], f32, tag="bc", name="bc")
                        nc.gpsimd.partition_broadcast(bc[0:D, :], rc[0:1, :], channels=D)
                        if debug_taps and qc == 0 and h == 0:
                            nc.sync.dma_start(out=dbg["bc0"][:, :], in_=bc)
                        if r0 == 0:
                            nc.vector.tensor_mul(
                                ot_t[m][qc][0:D, :], o_ps[h][0:D, :], bc[0:D, :])
                        else:
                            stg = sm.tile([P, NQ], bf16, tag="stg", name="stg")
                            nc.vector.tensor_mul(stg[0:D, :], o_ps[h][0:D, :], bc[0:D, :])
                            nc.sync.dma_start(out=ot_t[m][qc][D:P, :], in_=stg[0:D, :])

                    alloc()
                    for ktb in range(0, nkt, 2):
                        yield (lambda ktb=ktb, batch=batch: batch(ktb))
                    for h in pair:
                        yield (lambda h=h, norm=norm: norm(h))

            def wo_split_closures(qc):
                # last chunk: m0 half runs early (needs only pair (0,1)'s
                # normalized O^T), m1 half + add + store is the short tail
                so_sp = [ost.tile([P, NQ], f32, tag=f"so_sp{i}", name=f"so_sp{i}",
                                  bufs=1)
                         for i in range(8)]

                def m0_group(i, ti, ec):
                    def f():
                        wp0 = pp.tile([P, NQ], f32, tag="pp_t", name="wp0")
                        nc.tensor.matmul(
                            wp0,
                            lhsT=ot_t[0][qc][:, (ti % 4) * P:(ti % 4 + 1) * P],
                            rhs=wo_sb[:, 0, ec * NQ:(ec + 1) * NQ],
                            start=True, stop=True,
                        )
                        if ec % 2 == 0:
                            nc.scalar.copy(out=so_sp[i], in_=wp0)
                        else:
                            nc.vector.tensor_copy(out=so_sp[i], in_=wp0)
                    return f

                def m1_group(i, ti, ec):
                    def f():
                        wp1 = pp.tile([P, NQ], f32, tag="pp_t", name="wp1")
                        nc.tensor.matmul(
                            wp1,
                            lhsT=ot_t[1][qc][:, (ti % 4) * P:(ti % 4 + 1) * P],
                            rhs=wo_sb[:, 1, ec * NQ:(ec + 1) * NQ],
                            start=True, stop=True,
                        )
                        nc.vector.tensor_add(out=so_sp[i], in0=so_sp[i], in1=wp1)
                        nc.sync.dma_start(
                            out=out[ti * P:(ti + 1) * P, ec * NQ:(ec + 1) * NQ],
                            in_=so_sp[i])
                    return f

                tis = [(ti, ec) for ti in range(4 * qc, 4 * qc + 4)
                       for ec in range(E // NQ)]
                m0s = [m0_group(i, ti, ec) for i, (ti, ec) in enumerate(tis)]
                m1s = [m1_group(i, ti, ec) for i, (ti, ec) in enumerate(tis)]
                return m0s, m1s

            for f in proj_closures(0):
                f()
            last = QC - 1
            for qc in range(QC):
                slots = list(attn_stream(qc))
                early = []
                if qc + 1 < QC:
                    early += proj_closures(qc + 1)
                if 1 <= qc < last:
                    early += wo_closures(qc - 1)
                if qc == last:
                    early += wo_closures(qc - 1)
                    m0s, m1s = wo_split_closures(qc)
                else:
                    m0s, m1s = [], []
                # spread `early` over the first half of the slots, m0s over
                # the second half (after pair (0,1) has normalized)
                half = len(slots) // 2
                lists = [(early, slots[:half] or slots), (m0s, slots[half:] or slots)]
                plan = {i: [] for i in range(len(slots))}
                for fl, sl in lists:
                    base = slots.index(sl[0]) if sl else 0
                    n, s = len(fl), len(sl)
                    fi = 0
                    for i in range(s):
                        want = (i + 1) * n // s
                        while fi < want:
                            plan[base + i].append(fl[fi])
                            fi += 1
                for i, sf in enumerate(slots):
                    sf()
                    for f in plan[i]:
                        f()
                for f in m1s:
                    f()
            
            if debug_taps:
                nc.sync.dma_start(out=dbg["qt00"][:, :], in_=qt_t[0][0])
                nc.sync.dma_start(out=dbg["kt00"][:, :], in_=kt_t[0][0])
                nc.sync.dma_start(out=dbg["vo0"][:, :],
                                  in_=vo_t[0].rearrange("p h d -> p (h d)"))
                nc.sync.dma_start(out=dbg["ot00"][:, :], in_=ot_t[0][0])
    nc.compile()
    return nc


def _prepare_in_maps(x, Wq, bq, Wk, Wv, Wo):
    import ml_dtypes
    bfd = ml_dtypes.bfloat16
    tri = np.triu(np.ones((P, P), np.float32)).astype(bfd)
    xtb = [np.ascontiguousarray(x[b].T).astype(bfd) for b in range(B)]
    in_maps = []
    for c in range(NCORES):
        b, g = c // G, c % G
        cs = slice(g * CW, (g + 1) * CW)
        bq_g = np.ascontiguousarray(bq[cs].reshape(MT, P).T)
        def wlay(w):  # [E, CW] -> [P, ET, CW] with e = a*P + p
            return np.ascontiguousarray(
                w.reshape(ET, P, CW).transpose(1, 0, 2)).astype(bfd)
        wo_l = np.ascontiguousarray(
            Wo[cs, :].reshape(MT, P, E).transpose(1, 0, 2)).astype(bfd)
        in_maps.append({
            "xt": xtb[b],
            "wq": wlay(Wq[:, cs]),
            "wk": wlay(Wk[:, cs]),
            "wv": wlay(Wv[:, cs]),
            "bq": bq_g,
            "wo": wo_l,
            "tri": tri,
        })
    return in_maps


def _run(inputs, trace=False, trace_kwargs=None, debug_taps=False):
    from concourse.bass_utils import run_bass_kernel_spmd

    key = ("nc", debug_taps)
    if key not in _CACHE:
        _CACHE[key] = _build_bass(debug_taps=debug_taps)
    nc = _CACHE[key]

    x = np.asarray(inputs["x"], np.float32)
    Wq = np.asarray(inputs["Wq"], np.float32)
    Wk = np.asarray(inputs["Wk"], np.float32)
    Wv = np.asarray(inputs["Wv"], np.float32)
    Wo = np.asarray(inputs["Wo"], np.float32)
    bq = np.asarray(inputs["bq"], np.float32)
    bv = np.asarray(inputs["bv"], np.float32)
    bo = np.asarray(inputs["bo"], np.float32)

    in_maps = _prepare_in_maps(x, Wq, bq, Wk, Wv, Wo)
    res = run_bass_kernel_spmd(
        nc, in_maps, core_ids=list(range(NCORES)),
        trace=trace, **(trace_kwargs or {}))

    bias_row = (bv @ Wo + bo).astype(np.float32)
    y = np.empty((B, T, E), np.float32)
    for b in range(B):
        acc = res.results[G * b]["out"].astype(np.float32).copy()
        for g in range(1, G):
            acc += res.results[G * b + g]["out"]
        y[b] = acc + bias_row
    return y, res


def kernel(**inputs) -> np.ndarray:
    return _run(inputs, trace=False)[0]


# revision 35
# speedup vs baseline: 1.0354x; 1.0354x over previous
"""Causal multi-head attention layer for Trainium2, sharded over 8 NeuronCores.

Problem: B=2, T=2048, E=1024, H=16 heads (D=64), fp32.
  out = softmax(mask(QK^T)/sqrt(E)) V Wo + bo   with Q=xWq+bq etc.

Sharding: data-parallel over batch (2) x tensor-parallel over head groups (4):
core c -> batch b=c//4, head group g=c%4 (4 heads, 256 channels).
Each core computes partial = attn_heads(x_b) @ Wo[rows of g]; host sums the 4
partials per batch and adds the bias row.

Math folds used (all exact):
 - bk drops out of softmax (additive shift along the softmax axis).
 - attn rows sum to 1  =>  attn @ (V + 1 bv^T) = attn@V + 1 bv^T, so bv enters
   the output as the constant row bv @ Wo, added on the host with bo.
 - bq is added to Q^T on-chip (per-partition bias).

Device layout per core (all matmuls out = lhsT.T @ rhs, contraction on
partitions):
 - host passes x_b^T; projections: QT/KT = W^T x^T (lhsT=W tiles), V = x W
   (lhsT = x^T tiles), all in fp32r (full-rate fp32 matmul mode).
 - energy^T(kt-block, q) = K Q^T per head, two heads row-packed in the
   128x128 PE array (d=64 contraction at base partitions 0/64).
 - U = exp(energy^T/32) on ACT straight out of PSUM, bf16, causal blocks
   only; diagonal blocks masked with a 128x128 triangular multiply.
 - O^T = [V | 1]^T U accumulated over kt in PSUM: rows 0..63 = unnormalized
   output, row 64 = softmax denominator (free).
 - normalize with DVE mult by gpsimd-broadcast reciprocal; odd heads are
   shifted to partitions 64..127 of the O^T store by an SBUF->SBUF DMA.
 - partial = (O^T).T @ Wo rows, PSUM -> SBUF -> DRAM.
"""

import os
import numpy as np

B, T, E, H = 2, 2048, 1024, 16
P = 128
NCORES = 8
G = 4            # head groups (tensor parallel)
HG = H // G      # heads per group = 4
D = E // H       # 64
CW = HG * D      # channels per group = 256
ET = E // P      # 8 e-tiles
MT = CW // P     # 2 hd-tiles
TT = T // P      # 16 t-tiles
NQ = 512         # q-chunk width
QC = T // NQ     # 4 q-chunks
SCALE = 1.0 / np.sqrt(E).astype(np.float32)  # 1/32

_CACHE: dict = {}


def _build_bass(debug_taps=False):
    import concourse.bass as bass
    import concourse.mybir as mybir
    import concourse.tile as tile
    from concourse import bacc

    f32 = mybir.dt.float32
    f32r = mybir.dt.float32r  # fp32 data, full-rate (1 cyc/row) PE matmul mode
    bf16 = mybir.dt.bfloat16
    Exp = mybir.ActivationFunctionType.Exp

    nc = bacc.Bacc("TRN2", target_bir_lowering=False, name="attn_tp")
    dbg = {}
    if debug_taps:
        dbg["qt00"] = nc.dram_tensor("dbg_qt00", [P, NQ], bf16, kind="ExternalOutput")
        dbg["kt00"] = nc.dram_tensor("dbg_kt00", [P, NQ], bf16, kind="ExternalOutput")
        dbg["vo0"] = nc.dram_tensor("dbg_vo0", [P, HG * (D + 1)], bf16, kind="ExternalOutput")
        dbg["u000"] = nc.dram_tensor("dbg_u000", [P, 2 * NQ], bf16, kind="ExternalOutput")
        dbg["oun0"] = nc.dram_tensor("dbg_oun0", [P, NQ], f32, kind="ExternalOutput")
        dbg["bc0"] = nc.dram_tensor("dbg_bc0", [P, NQ], f32, kind="ExternalOutput")
        dbg["ot00"] = nc.dram_tensor("dbg_ot00", [P, NQ], bf16, kind="ExternalOutput")
    xt = nc.dram_tensor("xt", [E, T], bf16, kind="ExternalInput")
    wq = nc.dram_tensor("wq", [P, ET, CW], bf16, kind="ExternalInput")
    wk = nc.dram_tensor("wk", [P, ET, CW], bf16, kind="ExternalInput")
    wv = nc.dram_tensor("wv", [P, ET, CW], bf16, kind="ExternalInput")
    bq = nc.dram_tensor("bq", [P, MT], f32, kind="ExternalInput")
    wo = nc.dram_tensor("wo", [P, MT, E], bf16, kind="ExternalInput")
    tri = nc.dram_tensor("tri", [P, P], bf16, kind="ExternalInput")
    out = nc.dram_tensor("out", [T, E], f32, kind="ExternalOutput")

    with tile.TileContext(nc) as tc:
        with (
            tc.tile_pool(name="persist", bufs=1) as pers,
            tc.tile_pool(name="pp", bufs=2, space="PSUM") as pp,
            tc.tile_pool(name="ep", bufs=2, space="PSUM") as ep,
            tc.tile_pool(name="op", bufs=2, space="PSUM") as op,
            tc.tile_pool(name="up", bufs=8) as up,
            tc.tile_pool(name="sm", bufs=4) as sm,
            tc.tile_pool(name="ost", bufs=6) as ost,
        ):
            # ---- persistent SBUF tensors ----
            xt_t = [pers.tile([P, T], bf16, tag=f"xt{a}", name=f"xt{a}") for a in range(ET)]
            wq_sb = pers.tile([P, ET, CW], bf16, tag="wq_sb", name="wq_sb")
            wk_sb = pers.tile([P, ET, CW], bf16, tag="wk_sb", name="wk_sb")
            wv_sb = pers.tile([P, ET, CW], bf16, tag="wv_sb", name="wv_sb")
            wo_sb = pers.tile([P, MT, E], bf16, tag="wo_sb", name="wo_sb")
            bq_sb = pers.tile([P, MT], f32, tag="bq_sb", name="bq_sb")
            tri_sb = pers.tile([P, P], bf16, tag="tri_sb", name="tri_sb")
            qt_t = [[pers.tile([P, NQ], bf16, tag=f"qt{m}_{n}", name=f"qt{m}_{n}")
                     for n in range(QC)] for m in range(MT)]
            kt_t = [[pers.tile([P, NQ], bf16, tag=f"kt{m}_{n}", name=f"kt{m}_{n}")
                     for n in range(QC)] for m in range(MT)]
            vo_t = [pers.tile([P, HG, D + 1], bf16, tag=f"vo{t}", name=f"vo{t}")
                    for t in range(TT)]
            ot_t = [[pers.tile([P, NQ], bf16, tag=f"ot{m}_{n}", name=f"ot{m}_{n}")
                     for n in range(QC)] for m in range(MT)]

            # ---- input DMAs (weights first so projections start early) ----
            nc.scalar.dma_start(out=wq_sb, in_=wq[:, :, :])
            nc.scalar.dma_start(out=wk_sb, in_=wk[:, :, :])
            nc.scalar.dma_start(out=bq_sb, in_=bq[:, :])
            nc.scalar.dma_start(out=wv_sb, in_=wv[:, :, :])
            nc.scalar.dma_start(out=tri_sb, in_=tri[:, :])
            xt_r = xt.rearrange("(a p) t -> a p t", p=P)
            for a in range(ET):
                nc.sync.dma_start(out=xt_t[a], in_=xt_r[a])
            nc.scalar.dma_start(out=wo_sb, in_=wo[:, :, :])

            # ---- software-pipelined emission ----
            # PE engine queues are in-order, so attention batches (gated on
            # ACT exp) are interleaved with independent filler work: the next
            # chunk's projection groups and the previous chunk's Wo groups.

            def proj_closures(n):
                def qk_group(wsb, m, biased):
                    def f():
                        ps = pp.tile([P, NQ], f32, tag="pp_t", name="psqk")
                        for a in range(ET):
                            nc.tensor.matmul(
                                ps,
                                lhsT=wsb[:, a, m * P:(m + 1) * P],
                                rhs=xt_t[a][:, n * NQ:(n + 1) * NQ],
                                start=(a == 0), stop=(a == ET - 1),
                            )
                        if biased:
                            nc.vector.tensor_scalar_add(
                                out=qt_t[m][n], in0=ps, scalar1=bq_sb[:, m:m + 1])
                        else:
                            nc.vector.tensor_copy(out=kt_t[m][n], in_=ps)
                    return f

                def v_group(t):
                    def f():
                        psv = pp.tile([P, NQ], f32, tag="pp_t", name="psv")
                        for a in range(ET):
                            nc.tensor.matmul(
                                psv[:, :CW],
                                lhsT=xt_t[a][:, t * P:(t + 1) * P],
                                rhs=wv_sb[:, a, :],
                                start=(a == 0), stop=(a == ET - 1),
                            )
                        nc.vector.tensor_copy(
                            out=vo_t[t][:, :, 0:D],
                            in_=psv[:, :CW].rearrange("p (h d) -> p h d", h=HG))
                        nc.vector.memset(vo_t[t][:, :, D:D + 1], 1.0)
                    return f

                fs = []
                for m in range(MT):
                    fs.append(qk_group(wq_sb, m, True))
                for m in range(MT):
                    fs.append(qk_group(wk_sb, m, False))
                for t in range(4 * n, 4 * n + 4):
                    fs.append(v_group(t))
                return fs

            def wo_closures(qc):
                def wo_group(ti, ec):
                    def f():
                        wp = pp.tile([P, NQ], f32, tag="pp_t", name="wp")
                        for m in range(MT):
                            nc.tensor.matmul(
                                wp,
                                lhsT=ot_t[m][qc][:, (ti % 4) * P:(ti % 4 + 1) * P],
                                rhs=wo_sb[:, m, ec * NQ:(ec + 1) * NQ],
                                start=(m == 0), stop=(m == MT - 1),
                            )
                        so = ost.tile([P, NQ], f32, tag="ost", name="so")
                        if ec % 2 == 0:
                            nc.scalar.copy(out=so, in_=wp)
                        else:
                            nc.vector.tensor_copy(out=so, in_=wp)
                        nc.sync.dma_start(
                            out=out[ti * P:(ti + 1) * P, ec * NQ:(ec + 1) * NQ], in_=so)
                    return f
                return [wo_group(ti, ec)
                        for ti in range(4 * qc, 4 * qc + 4) for ec in range(E // NQ)]

            def attn_stream(qc):
                nkt = 4 * qc + 4
                for pair in ((0, 1), (2, 3)):
                    o_ps = {}

                    def alloc(pair=pair, o_ps=o_ps):
                        for h in pair:
                            o_ps[h] = op.tile([P, NQ], f32, tag="o_ps", name=f"o_ps{h}")

                    def batch(ktb, pair=pair, o_ps=o_ps):
                        kts = (ktb, ktb + 1)
                        offs = [max(0, (kt - 4 * qc) * P) for kt in kts]
                        e_ts = {}
                        u_ts = {}
                        for h in pair:
                            e_ts[h] = ep.tile([P, 2 * NQ], f32, tag="e_ps",
                                              name=f"e_ps{h}")
                        # alternate heads so adjacent matmuls use disjoint PE
                        # row groups (base partitions 0/64): LDWEIGHTS of the
                        # next matmul overlaps the in-flight one
                        for j, kt in enumerate(kts):
                            eoff = offs[j]
                            for h in pair:
                                m, r0 = h // 2, 64 * (h % 2)
                                nc.tensor.matmul(
                                    e_ts[h][:, j * NQ + eoff:(j + 1) * NQ],
                                    lhsT=kt_t[m][kt // 4][r0:r0 + D,
                                                          (kt % 4) * P:(kt % 4 + 1) * P],
                                    rhs=qt_t[m][qc][r0:r0 + D, eoff:NQ],
                                    start=True, stop=True,
                                )
                        for h in pair:
                            ut = up.tile([P, 2 * NQ], bf16, tag="u", name=f"u{h}")
                            u_ts[h] = ut
                            if offs[1] <= P:
                                # single ACTIVATE; the [NQ, NQ+off1) hole is
                                # never read downstream
                                nc.scalar.activation(
                                    ut[:, offs[0]:], e_ts[h][:, offs[0]:],
                                    Exp, scale=float(SCALE))
                            else:
                                for j, off in enumerate(offs):
                                    nc.scalar.activation(
                                        ut[:, j * NQ + off:(j + 1) * NQ],
                                        e_ts[h][:, j * NQ + off:(j + 1) * NQ],
                                        Exp, scale=float(SCALE))
                            for j, kt in enumerate(kts):
                                if kt >= 4 * qc:
                                    w0 = j * NQ + offs[j]
                                    nc.vector.tensor_mul(
                                        ut[:, w0:w0 + P], ut[:, w0:w0 + P], tri_sb)
                        if debug_taps and qc == 0 and pair == (0, 1) and ktb == 0:
                            nc.sync.dma_start(out=dbg["u000"][:, :], in_=u_ts[0])
                        for h in pair:
                            for j, kt in enumerate(kts):
                                off = offs[j]
                                nc.tensor.matmul(
                                    o_ps[h][0:D + 1, off:NQ],
                                    lhsT=vo_t[kt][:, h, :],
                                    rhs=u_ts[h][:, j * NQ + off:(j + 1) * NQ],
                                    start=(kt == 0), stop=(kt == nkt - 1),
                                )

                    def norm(h, pair=pair, o_ps=o_ps):
                        m, r0 = h // 2, 64 * (h % 2)
                        if debug_taps and qc == 0 and h == 0:
                            ou = sm.tile([P, NQ], f32, tag="oun", name="oun")
                            nc.vector.tensor_copy(out=ou[0:D + 1, :], in_=o_ps[h][0:D + 1, :])
                            nc.sync.dma_start(out=dbg["oun0"][:, :], in_=ou)
                        dn = sm.tile([P, NQ], f32, tag="dn", name="dn")
                        nc.vector.tensor_copy(out=dn[D:D + 1, :], in_=o_ps[h][D:D + 1, :])
                        nc.sync.dma_start(out=dn[0:1, :], in_=dn[D:D + 1, :])
                        rc = sm.tile([P, NQ], f32, tag="rc", name="rc")
                        nc.vector.reciprocal_approx_fast(out=rc[0:1, :], in_=dn[0:1, :])
                        bc = sm.tile([P, NQ], f32, tag="bc", name="bc")
                        nc.gpsimd.partition_broadcast(bc[0:D, :], rc[0:1, :], channels=D)
                        if debug_taps and qc == 0 and h == 0:
                            nc.sync.dma_start(out=dbg["bc0"][:, :], in_=bc)
                        if r0 == 0:
                            nc.vector.tensor_mul(
                                ot_t[m][qc][0:D, :], o_ps[h][0:D, :], bc[0:D, :])
                        else:
                            stg = sm.tile([P, NQ], bf16, tag="stg", name="stg")
                            nc.vector.tensor_mul(stg[0:D, :], o_ps[h][0:D, :], bc[0:D, :])
                            nc.sync.dma_start(out=ot_t[m][qc][D:P, :], in_=stg[0:D, :])

                    alloc()
                    for ktb in range(0, nkt, 2):
                        yield (lambda ktb=ktb, batch=batch: batch(ktb))
                    for h in pair:
                        yield (lambda h=h, norm=norm: norm(h))

            for f in proj_closures(0):
                f()
            for qc in range(QC):
                fillers = []
                if qc + 1 < QC:
                    fillers += proj_closures(qc + 1)
                if qc >= 1:
                    fillers += wo_closures(qc - 1)
                slots = list(attn_stream(qc))
                reserve = min(3, len(fillers))
                spread = fillers[:len(fillers) - reserve]
                nf, ns, fi = len(spread), len(slots), 0
                for i, sf in enumerate(slots):
                    sf()
                    want = (i + 1) * nf // ns
                    while fi < want:
                        spread[fi]()
                        fi += 1
                for f in fillers[len(fillers) - reserve:]:
                    f()
            for f in wo_closures(QC - 1):
                f()
            if debug_taps:
                nc.sync.dma_start(out=dbg["qt00"][:, :], in_=qt_t[0][0])
                nc.sync.dma_start(out=dbg["kt00"][:, :], in_=kt_t[0][0])
                nc.sync.dma_start(out=dbg["vo0"][:, :],
                                  in_=vo_t[0].rearrange("p h d -> p (h d)"))
                nc.sync.dma_start(out=dbg["ot00"][:, :], in_=ot_t[0][0])
    nc.compile()
    return nc


def _prepare_in_maps(x, Wq, bq, Wk, Wv, Wo):
    import ml_dtypes
    bfd = ml_dtypes.bfloat16
    tri = np.triu(np.ones((P, P), np.float32)).astype(bfd)
    xtb = [np.ascontiguousarray(x[b].T).astype(bfd) for b in range(B)]
    in_maps = []
    for c in range(NCORES):
        b, g = c // G, c % G
        cs = slice(g * CW, (g + 1) * CW)
        bq_g = np.ascontiguousarray(bq[cs].reshape(MT, P).T)
        def wlay(w):  # [E, CW] -> [P, ET, CW] with e = a*P + p
            return np.ascontiguousarray(
                w.reshape(ET, P, CW).transpose(1, 0, 2)).astype(bfd)
        wo_l = np.ascontiguousarray(
            Wo[cs, :].reshape(MT, P, E).transpose(1, 0, 2)).astype(bfd)
        in_maps.append({
            "xt": xtb[b],
            "wq": wlay(Wq[:, cs]),
            "wk": wlay(Wk[:, cs]),
            "wv": wlay(Wv[:, cs]),
            "bq": bq_g,
            "wo": wo_l,
            "tri": tri,
        })
    return in_maps


def _run(inputs, trace=False, trace_kwargs=None, debug_taps=False):
    from concourse.bass_utils import run_bass_kernel_spmd

    key = ("nc", debug_taps)
    if key not in _CACHE:
        _CACHE[key] = _build_bass(debug_taps=debug_taps)
    nc = _CACHE[key]

    x = np.asarray(inputs["x"], np.float32)
    Wq = np.asarray(inputs["Wq"], np.float32)
    Wk = np.asarray(inputs["Wk"], np.float32)
    Wv = np.asarray(inputs["Wv"], np.float32)
    Wo = np.asarray(inputs["Wo"], np.float32)
    bq = np.asarray(inputs["bq"], np.float32)
    bv = np.asarray(inputs["bv"], np.float32)
    bo = np.asarray(inputs["bo"], np.float32)

    in_maps = _prepare_in_maps(x, Wq, bq, Wk, Wv, Wo)
    res = run_bass_kernel_spmd(
        nc, in_maps, core_ids=list(range(NCORES)),
        trace=trace, **(trace_kwargs or {}))

    bias_row = (bv @ Wo + bo).astype(np.float32)
    y = np.empty((B, T, E), np.float32)
    for b in range(B):
        acc = res.results[G * b]["out"].astype(np.float32).copy()
        for g in range(1, G):
            acc += res.results[G * b + g]["out"]
        y[b] = acc + bias_row
    return y, res


def kernel(**inputs) -> np.ndarray:
    return _run(inputs, trace=False)[0]


# revision 36
# speedup vs baseline: 1.0585x; 1.0223x over previous
"""Causal multi-head attention layer for Trainium2, sharded over 8 NeuronCores.

Problem: B=2, T=2048, E=1024, H=16 heads (D=64), fp32.
  out = softmax(mask(QK^T)/sqrt(E)) V Wo + bo   with Q=xWq+bq etc.

Sharding: data-parallel over batch (2) x tensor-parallel over head groups (4):
core c -> batch b=c//4, head group g=c%4 (4 heads, 256 channels).
Each core computes partial = attn_heads(x_b) @ Wo[rows of g]; host sums the 4
partials per batch and adds the bias row.

Math folds used (all exact):
 - bk drops out of softmax (additive shift along the softmax axis).
 - attn rows sum to 1  =>  attn @ (V + 1 bv^T) = attn@V + 1 bv^T, so bv enters
   the output as the constant row bv @ Wo, added on the host with bo.
 - bq is added to Q^T on-chip (per-partition bias).

Device layout per core (all matmuls out = lhsT.T @ rhs, contraction on
partitions):
 - host passes x_b^T; projections: QT/KT = W^T x^T (lhsT=W tiles), V = x W
   (lhsT = x^T tiles), all in fp32r (full-rate fp32 matmul mode).
 - energy^T(kt-block, q) = K Q^T per head, two heads row-packed in the
   128x128 PE array (d=64 contraction at base partitions 0/64).
 - U = exp(energy^T/32) on ACT straight out of PSUM, bf16, causal blocks
   only; diagonal blocks masked with a 128x128 triangular multiply.
 - O^T = [V | 1]^T U accumulated over kt in PSUM: rows 0..63 = unnormalized
   output, row 64 = softmax denominator (free).
 - normalize with DVE mult by gpsimd-broadcast reciprocal; odd heads are
   shifted to partitions 64..127 of the O^T store by an SBUF->SBUF DMA.
 - partial = (O^T).T @ Wo rows, PSUM -> SBUF -> DRAM.
"""

import os
import numpy as np

B, T, E, H = 2, 2048, 1024, 16
P = 128
NCORES = 8
G = 4            # head groups (tensor parallel)
HG = H // G      # heads per group = 4
D = E // H       # 64
CW = HG * D      # channels per group = 256
ET = E // P      # 8 e-tiles
MT = CW // P     # 2 hd-tiles
TT = T // P      # 16 t-tiles
NQ = 512         # q-chunk width
QC = T // NQ     # 4 q-chunks
SCALE = 1.0 / np.sqrt(E).astype(np.float32)  # 1/32

_CACHE: dict = {}


def _build_bass(debug_taps=False):
    import concourse.bass as bass
    import concourse.mybir as mybir
    import concourse.tile as tile
    from concourse import bacc

    f32 = mybir.dt.float32
    f32r = mybir.dt.float32r  # fp32 data, full-rate (1 cyc/row) PE matmul mode
    bf16 = mybir.dt.bfloat16
    Exp = mybir.ActivationFunctionType.Exp

    nc = bacc.Bacc("TRN2", target_bir_lowering=False, name="attn_tp")
    dbg = {}
    if debug_taps:
        dbg["qt00"] = nc.dram_tensor("dbg_qt00", [P, NQ], bf16, kind="ExternalOutput")
        dbg["kt00"] = nc.dram_tensor("dbg_kt00", [P, NQ], bf16, kind="ExternalOutput")
        dbg["vo0"] = nc.dram_tensor("dbg_vo0", [P, HG * (D + 1)], bf16, kind="ExternalOutput")
        dbg["u000"] = nc.dram_tensor("dbg_u000", [P, 2 * NQ], bf16, kind="ExternalOutput")
        dbg["oun0"] = nc.dram_tensor("dbg_oun0", [P, NQ], f32, kind="ExternalOutput")
        dbg["bc0"] = nc.dram_tensor("dbg_bc0", [P, NQ], f32, kind="ExternalOutput")
        dbg["ot00"] = nc.dram_tensor("dbg_ot00", [P, NQ], bf16, kind="ExternalOutput")
    xt = nc.dram_tensor("xt", [E, T], bf16, kind="ExternalInput")
    wq = nc.dram_tensor("wq", [P, ET, CW], bf16, kind="ExternalInput")
    wk = nc.dram_tensor("wk", [P, ET, CW], bf16, kind="ExternalInput")
    wv = nc.dram_tensor("wv", [P, ET, CW], bf16, kind="ExternalInput")
    bq = nc.dram_tensor("bq", [P, MT], f32, kind="ExternalInput")
    wo = nc.dram_tensor("wo", [P, MT, E], bf16, kind="ExternalInput")
    tri = nc.dram_tensor("tri", [P, P], bf16, kind="ExternalInput")
    out = nc.dram_tensor("out", [T, E], f32, kind="ExternalOutput")

    with tile.TileContext(nc) as tc:
        with (
            tc.tile_pool(name="persist", bufs=1) as pers,
            tc.tile_pool(name="pp", bufs=2, space="PSUM") as pp,
            tc.tile_pool(name="ep", bufs=2, space="PSUM") as ep,
            tc.tile_pool(name="op", bufs=2, space="PSUM") as op,
            tc.tile_pool(name="up", bufs=8) as up,
            tc.tile_pool(name="sm", bufs=4) as sm,
            tc.tile_pool(name="ost", bufs=6) as ost,
        ):
            # ---- persistent SBUF tensors ----
            xt_t = [pers.tile([P, T], bf16, tag=f"xt{a}", name=f"xt{a}") for a in range(ET)]
            wq_sb = pers.tile([P, ET, CW], bf16, tag="wq_sb", name="wq_sb")
            wk_sb = pers.tile([P, ET, CW], bf16, tag="wk_sb", name="wk_sb")
            wv_sb = pers.tile([P, ET, CW], bf16, tag="wv_sb", name="wv_sb")
            wo_sb = pers.tile([P, MT, E], bf16, tag="wo_sb", name="wo_sb")
            bq_sb = pers.tile([P, MT], f32, tag="bq_sb", name="bq_sb")
            tri_sb = pers.tile([P, P], bf16, tag="tri_sb", name="tri_sb")
            qt_t = [[pers.tile([P, NQ], bf16, tag=f"qt{m}_{n}", name=f"qt{m}_{n}")
                     for n in range(QC)] for m in range(MT)]
            kt_t = [[pers.tile([P, NQ], bf16, tag=f"kt{m}_{n}", name=f"kt{m}_{n}")
                     for n in range(QC)] for m in range(MT)]
            vo_t = [pers.tile([P, HG, D + 1], bf16, tag=f"vo{t}", name=f"vo{t}")
                    for t in range(TT)]
            ot_t = [[pers.tile([P, NQ], bf16, tag=f"ot{m}_{n}", name=f"ot{m}_{n}")
                     for n in range(QC)] for m in range(MT)]

            # ---- input DMAs (weights first so projections start early) ----
            nc.scalar.dma_start(out=wq_sb, in_=wq[:, :, :])
            nc.scalar.dma_start(out=bq_sb, in_=bq[:, :])
            nc.scalar.dma_start(out=wk_sb, in_=wk[:, :, :])
            nc.scalar.dma_start(out=wv_sb, in_=wv[:, :, :])
            nc.scalar.dma_start(out=tri_sb, in_=tri[:, :])
            xt_r = xt.rearrange("(a p) t -> a p t", p=P)
            for a in range(ET):
                nc.sync.dma_start(out=xt_t[a], in_=xt_r[a])
            nc.scalar.dma_start(out=wo_sb, in_=wo[:, :, :])

            # ---- software-pipelined emission ----
            # PE engine queues are in-order, so attention batches (gated on
            # ACT exp) are interleaved with independent filler work: the next
            # chunk's projection groups and the previous chunk's Wo groups.

            def proj_closures(n):
                def qk_group(wsb, m, biased):
                    def f():
                        ps = pp.tile([P, NQ], f32, tag="pp_t", name="psqk")
                        for a in range(ET):
                            nc.tensor.matmul(
                                ps,
                                lhsT=wsb[:, a, m * P:(m + 1) * P],
                                rhs=xt_t[a][:, n * NQ:(n + 1) * NQ],
                                start=(a == 0), stop=(a == ET - 1),
                            )
                        if biased:
                            nc.vector.tensor_scalar_add(
                                out=qt_t[m][n], in0=ps, scalar1=bq_sb[:, m:m + 1])
                        else:
                            nc.vector.tensor_copy(out=kt_t[m][n], in_=ps)
                    return f

                def v_group(t):
                    def f():
                        psv = pp.tile([P, NQ], f32, tag="pp_t", name="psv")
                        for a in range(ET):
                            nc.tensor.matmul(
                                psv[:, :CW],
                                lhsT=xt_t[a][:, t * P:(t + 1) * P],
                                rhs=wv_sb[:, a, :],
                                start=(a == 0), stop=(a == ET - 1),
                            )
                        nc.vector.tensor_copy(
                            out=vo_t[t][:, :, 0:D],
                            in_=psv[:, :CW].rearrange("p (h d) -> p h d", h=HG))
                        nc.vector.memset(vo_t[t][:, :, D:D + 1], 1.0)
                    return f

                fs = []
                for m in range(MT):
                    fs.append(qk_group(wq_sb, m, True))
                    fs.append(qk_group(wk_sb, m, False))
                for t in range(4 * n, 4 * n + 4):
                    fs.append(v_group(t))
                return fs

            def wo_closures(qc):
                def wo_group(ti, ec):
                    def f():
                        wp = pp.tile([P, NQ], f32, tag="pp_t", name="wp")
                        for m in range(MT):
                            nc.tensor.matmul(
                                wp,
                                lhsT=ot_t[m][qc][:, (ti % 4) * P:(ti % 4 + 1) * P],
                                rhs=wo_sb[:, m, ec * NQ:(ec + 1) * NQ],
                                start=(m == 0), stop=(m == MT - 1),
                            )
                        so = ost.tile([P, NQ], f32, tag="ost", name="so")
                        if ec % 2 == 0:
                            nc.scalar.copy(out=so, in_=wp)
                        else:
                            nc.vector.tensor_copy(out=so, in_=wp)
                        nc.sync.dma_start(
                            out=out[ti * P:(ti + 1) * P, ec * NQ:(ec + 1) * NQ], in_=so)
                    return f
                return [wo_group(ti, ec)
                        for ti in range(4 * qc, 4 * qc + 4) for ec in range(E // NQ)]

            def attn_stream(qc):
                nkt = 4 * qc + 4
                for pair in ((0, 1), (2, 3)):
                    o_ps = {}

                    def alloc(pair=pair, o_ps=o_ps):
                        for h in pair:
                            o_ps[h] = op.tile([P, NQ], f32, tag="o_ps", name=f"o_ps{h}")

                    def batch(ktb, pair=pair, o_ps=o_ps):
                        kts = (ktb, ktb + 1)
                        offs = [max(0, (kt - 4 * qc) * P) for kt in kts]
                        e_ts = {}
                        u_ts = {}
                        for h in pair:
                            e_ts[h] = ep.tile([P, 2 * NQ], f32, tag="e_ps",
                                              name=f"e_ps{h}")
                        # alternate heads so adjacent matmuls use disjoint PE
                        # row groups (base partitions 0/64): LDWEIGHTS of the
                        # next matmul overlaps the in-flight one
                        for j, kt in enumerate(kts):
                            eoff = offs[j]
                            for h in pair:
                                m, r0 = h // 2, 64 * (h % 2)
                                nc.tensor.matmul(
                                    e_ts[h][:, j * NQ + eoff:(j + 1) * NQ],
                                    lhsT=kt_t[m][kt // 4][r0:r0 + D,
                                                          (kt % 4) * P:(kt % 4 + 1) * P],
                                    rhs=qt_t[m][qc][r0:r0 + D, eoff:NQ],
                                    start=True, stop=True,
                                )
                        for h in pair:
                            ut = up.tile([P, 2 * NQ], bf16, tag="u", name=f"u{h}")
                            u_ts[h] = ut
                            if offs[1] <= P:
                                # single ACTIVATE; the [NQ, NQ+off1) hole is
                                # never read downstream
                                nc.scalar.activation(
                                    ut[:, offs[0]:], e_ts[h][:, offs[0]:],
                                    Exp, scale=float(SCALE))
                            else:
                                for j, off in enumerate(offs):
                                    nc.scalar.activation(
                                        ut[:, j * NQ + off:(j + 1) * NQ],
                                        e_ts[h][:, j * NQ + off:(j + 1) * NQ],
                                        Exp, scale=float(SCALE))
                            for j, kt in enumerate(kts):
                                if kt >= 4 * qc:
                                    w0 = j * NQ + offs[j]
                                    nc.vector.tensor_mul(
                                        ut[:, w0:w0 + P], ut[:, w0:w0 + P], tri_sb)
                        if debug_taps and qc == 0 and pair == (0, 1) and ktb == 0:
                            nc.sync.dma_start(out=dbg["u000"][:, :], in_=u_ts[0])
                        for h in pair:
                            for j, kt in enumerate(kts):
                                off = offs[j]
                                nc.tensor.matmul(
                                    o_ps[h][0:D + 1, off:NQ],
                                    lhsT=vo_t[kt][:, h, :],
                                    rhs=u_ts[h][:, j * NQ + off:(j + 1) * NQ],
                                    start=(kt == 0), stop=(kt == nkt - 1),
                                )

                    def norm(h, pair=pair, o_ps=o_ps):
                        m, r0 = h // 2, 64 * (h % 2)
                        if debug_taps and qc == 0 and h == 0:
                            ou = sm.tile([P, NQ], f32, tag="oun", name="oun")
                            nc.vector.tensor_copy(out=ou[0:D + 1, :], in_=o_ps[h][0:D + 1, :])
                            nc.sync.dma_start(out=dbg["oun0"][:, :], in_=ou)
                        dn = sm.tile([P, NQ], f32, tag="dn", name="dn")
                        nc.vector.tensor_copy(out=dn[D:D + 1, :], in_=o_ps[h][D:D + 1, :])
                        nc.sync.dma_start(out=dn[0:1, :], in_=dn[D:D + 1, :])
                        rc = sm.tile([P, NQ], f32, tag="rc", name="rc")
                        nc.vector.reciprocal_approx_fast(out=rc[0:1, :], in_=dn[0:1, :])
                        bc = sm.tile([P, NQ], f32, tag="bc", name="bc")
                        nc.gpsimd.partition_broadcast(bc[0:D, :], rc[0:1, :], channels=D)
                        if debug_taps and qc == 0 and h == 0:
                            nc.sync.dma_start(out=dbg["bc0"][:, :], in_=bc)
                        if r0 == 0:
                            nc.vector.tensor_mul(
                                ot_t[m][qc][0:D, :], o_ps[h][0:D, :], bc[0:D, :])
                        else:
                            stg = sm.tile([P, NQ], bf16, tag="stg", name="stg")
                            nc.vector.tensor_mul(stg[0:D, :], o_ps[h][0:D, :], bc[0:D, :])
                            nc.sync.dma_start(out=ot_t[m][qc][D:P, :], in_=stg[0:D, :])

                    alloc()
                    for ktb in range(0, nkt, 2):
                        yield (lambda ktb=ktb, batch=batch: batch(ktb))
                    for h in pair:
                        yield (lambda h=h, norm=norm: norm(h))

            for f in proj_closures(0):
                f()
            for qc in range(QC):
                fillers = []
                if qc + 1 < QC:
                    fillers += proj_closures(qc + 1)
                if qc >= 1:
                    fillers += wo_closures(qc - 1)
                slots = list(attn_stream(qc))
                reserve = min(3, len(fillers))
                spread = fillers[:len(fillers) - reserve]
                nf, ns, fi = len(spread), len(slots), 0
                for i, sf in enumerate(slots):
                    sf()
                    want = (i + 1) * nf // ns
                    while fi < want:
                        spread[fi]()
                        fi += 1
                for f in fillers[len(fillers) - reserve:]:
                    f()
            for f in wo_closures(QC - 1):
                f()
            if debug_taps:
                nc.sync.dma_start(out=dbg["qt00"][:, :], in_=qt_t[0][0])
                nc.sync.dma_start(out=dbg["kt00"][:, :], in_=kt_t[0][0])
                nc.sync.dma_start(out=dbg["vo0"][:, :],
                                  in_=vo_t[0].rearrange("p h d -> p (h d)"))
                nc.sync.dma_start(out=dbg["ot00"][:, :], in_=ot_t[0][0])
    nc.compile()
    return nc


def _prepare_in_maps(x, Wq, bq, Wk, Wv, Wo):
    import ml_dtypes
    bfd = ml_dtypes.bfloat16
    tri = np.triu(np.ones((P, P), np.float32)).astype(bfd)
    xtb = [np.ascontiguousarray(x[b].T).astype(bfd) for b in range(B)]
    in_maps = []
    for c in range(NCORES):
        b, g = c // G, c % G
        cs = slice(g * CW, (g + 1) * CW)
        bq_g = np.ascontiguousarray(bq[cs].reshape(MT, P).T)
        def wlay(w):  # [E, CW] -> [P, ET, CW] with e = a*P + p
            return np.ascontiguousarray(
                w.reshape(ET, P, CW).transpose(1, 0, 2)).astype(bfd)
        wo_l = np.ascontiguousarray(
            Wo[cs, :].reshape(MT, P, E).transpose(1, 0, 2)).astype(bfd)
        in_maps.append({
            "xt": xtb[b],
            "wq": wlay(Wq[:, cs]),
            "wk": wlay(Wk[:, cs]),
            "wv": wlay(Wv[:, cs]),
            "bq": bq_g,
            "wo": wo_l,
            "tri": tri,
        })
    return in_maps


def _run(inputs, trace=False, trace_kwargs=None, debug_taps=False):
    from concourse.bass_utils import run_bass_kernel_spmd

    key = ("nc", debug_taps)
    if key not in _CACHE:
        _CACHE[key] = _build_bass(debug_taps=debug_taps)
    nc = _CACHE[key]

    x = np.asarray(inputs["x"], np.float32)
    Wq = np.asarray(inputs["Wq"], np.float32)
    Wk = np.asarray(inputs["Wk"], np.float32)
    Wv = np.asarray(inputs["Wv"], np.float32)
    Wo = np.asarray(inputs["Wo"], np.float32)
    bq = np.asarray(inputs["bq"], np.float32)
    bv = np.asarray(inputs["bv"], np.float32)
    bo = np.asarray(inputs["bo"], np.float32)

    in_maps = _prepare_in_maps(x, Wq, bq, Wk, Wv, Wo)
    res = run_bass_kernel_spmd(
        nc, in_maps, core_ids=list(range(NCORES)),
        trace=trace, **(trace_kwargs or {}))

    bias_row = (bv @ Wo + bo).astype(np.float32)
    y = np.empty((B, T, E), np.float32)
    for b in range(B):
        acc = res.results[G * b]["out"].astype(np.float32).copy()
        for g in range(1, G):
            acc += res.results[G * b + g]["out"]
        y[b] = acc + bias_row
    return y, res


def kernel(**inputs) -> np.ndarray:
    return _run(inputs, trace=False)[0]
